# revision 11
# baseline (speedup 1.0000x reference)
import numpy as np

B, P, D = 2048, 4, 128
N = B * P
NCLS = 32
N_CORES = 8
RPC = N // N_CORES          # 1024 rows per core
NRB = RPC // 128            # 8 row-blocks of 128
BIG = 1e30
EPS = 1e-12
MARGIN_SASC = 0.3
MARGIN_SADC = 0.2
MARGIN_DASC = 0.4


def _build_bass(S, W):
    import concourse.bass as bass
    from concourse import bacc, tile
    from concourse import mybir

    nc = bacc.Bacc("TRN2", target_bir_lowering=False, debug=False,
                   num_devices=N_CORES)

    f32 = mybir.dt.float32
    f32r = mybir.dt.float32r

    lhsT_d = nc.dram_tensor("lhsT", [D, RPC], f32r, kind="ExternalInput").ap()
    rhsT_d = nc.dram_tensor("rhsT", [D, W], f32r, kind="ExternalInput").ap()
    sqc_d = nc.dram_tensor("sqc", [1, W], f32r, kind="ExternalInput").ap()
    bigM_d = nc.dram_tensor("bigM", [128, NRB * NCLS * P], f32,
                            kind="ExternalInput").ap()
    negMnot_d = nc.dram_tensor("negMnot", [128, NRB * NCLS * P], f32,
                               kind="ExternalInput").ap()
    negMat_d = nc.dram_tensor("negMat", [128, NRB * NCLS * P], f32,
                              kind="ExternalInput").ap()
    pmask_d = nc.dram_tensor("pmask", [128, 16], f32,
                             kind="ExternalInput").ap()
    ones_d = nc.dram_tensor("ones", [1, 128], f32r,
                            kind="ExternalInput").ap()
    out_d = nc.dram_tensor("out", [128, NRB * 5], f32,
                           kind="ExternalOutput").ap()

    NSEG = P * NCLS          # 128 column segments of size S
    kmax = 512 // S          # segments per PSUM tile (fp32 PSUM = 512/part)
    n_tiles = (NSEG + kmax - 1) // kmax

    from contextlib import ExitStack
    with tile.TileContext(nc) as tc:
        with ExitStack() as ctx:
            const = ctx.enter_context(tc.tile_pool(name="const", bufs=1))
            valp = ctx.enter_context(tc.tile_pool(name="val", bufs=1))
            psum = ctx.enter_context(
                tc.tile_pool(name="psum", bufs=2, space="PSUM"))
            small = ctx.enter_context(tc.tile_pool(name="small", bufs=2))

            lhsT_sb = const.tile([D, RPC], f32r, tag="lhsT")
            rhsT_sb = const.tile([D, W], f32r, tag="rhsT")
            sqc_sb = const.tile([1, W], f32r, tag="sqc")
            bigM_sb = const.tile([128, NRB * NSEG], f32, tag="bigM")
            negMnot_sb = const.tile([128, NRB * NSEG], f32, tag="negMnot")
            negMat_sb = const.tile([128, NRB * NSEG], f32, tag="negMat")
            pmask_sb = const.tile([128, 16], f32, tag="pmask")
            ones_sb = const.tile([1, 128], f32r, tag="ones")
            out_sb = const.tile([128, NRB * 5], f32, tag="out")

            nc.gpsimd.dma_start(lhsT_sb[:], lhsT_d[:])
            nc.gpsimd.dma_start(rhsT_sb[:], rhsT_d[:])
            nc.gpsimd.dma_start(sqc_sb[:], sqc_d[:])
            nc.gpsimd.dma_start(bigM_sb[:], bigM_d[:])
            nc.gpsimd.dma_start(negMnot_sb[:], negMnot_d[:])
            nc.gpsimd.dma_start(negMat_sb[:], negMat_d[:])
            nc.gpsimd.dma_start(pmask_sb[:], pmask_d[:])
            nc.gpsimd.dma_start(ones_sb[:], ones_d[:])

            for rb in range(NRB):
                val = valp.tile([128, NSEG * S], f32, tag="val")
                val3 = val.rearrange("p (g s) -> p g s", g=NSEG)
                lhs_rb = lhsT_sb[:, rb * 128:(rb + 1) * 128]
                for t in range(n_tiles):
                    seg0 = t * kmax
                    k = min(kmax, NSEG - seg0)
                    nf = k * S
                    ps = psum.tile([128, kmax * S], f32, tag="ps",
                                   space="PSUM")
                    nc.tensor.matmul(ps[:, :nf], lhs_rb,
                                     rhsT_sb[:, seg0 * S:seg0 * S + nf],
                                     start=True, stop=False)
                    nc.tensor.matmul(ps[:, :nf], ones_sb,
                                     sqc_sb[:, seg0 * S:seg0 * S + nf],
                                     start=False, stop=True)
                    nc.scalar.copy(val[:, seg0 * S:seg0 * S + nf],
                                   ps[:, :nf])

                # segment reductions: [128, NSEG, S] -> [128, NSEG]
                segMin = small.tile([128, NSEG], f32, tag="segMin")
                segMax = small.tile([128, NSEG], f32, tag="segMax")
                nc.vector.tensor_reduce(segMin[:], val3,
                                        axis=mybir.AxisListType.X,
                                        op=mybir.AluOpType.min)
                nc.vector.tensor_reduce(segMax[:], val3,
                                        axis=mybir.AxisListType.X,
                                        op=mybir.AluOpType.max)

                mslice = slice(rb * NSEG, (rb + 1) * NSEG)
                sm3 = segMin.rearrange("p (q c) -> p q c", q=P)
                sx3 = segMax.rearrange("p (q c) -> p q c", q=P)

                # ---- mins ----
                tmp = small.tile([128, NSEG], f32, tag="tmp")
                nc.vector.tensor_add(tmp[:], segMin[:], bigM_sb[:, mslice])
                minD = small.tile([128, P], f32, tag="minD")
                nc.vector.tensor_reduce(
                    minD[:], tmp.rearrange("p (q c) -> p q c", q=P),
                    axis=mybir.AxisListType.X, op=mybir.AluOpType.min)
                minAll = small.tile([128, P], f32, tag="minAll")
                nc.vector.tensor_reduce(minAll[:], sm3,
                                        axis=mybir.AxisListType.X,
                                        op=mybir.AluOpType.min)

                t4 = small.tile([128, P], f32, tag="t4")
                c0 = rb * 5
                # an_sadc = min over q!=p of minD  (add +BIG at own part)
                nc.vector.tensor_add(t4[:], minD[:], pmask_sb[:, 0:4])
                nc.vector.tensor_reduce(out_sb[:, c0 + 3:c0 + 4], t4[:],
                                        axis=mybir.AxisListType.X,
                                        op=mybir.AluOpType.min)
                # an_sasc = min( minD at own part , minAll at other parts )
                t4b = small.tile([128, P], f32, tag="t4b")
                nc.vector.tensor_add(t4[:], minD[:], pmask_sb[:, 4:8])
                nc.vector.tensor_add(t4b[:], minAll[:], pmask_sb[:, 0:4])
                nc.vector.tensor_tensor(t4[:], t4[:], t4b[:],
                                        op=mybir.AluOpType.min)
                nc.vector.tensor_reduce(out_sb[:, c0 + 1:c0 + 2], t4[:],
                                        axis=mybir.AxisListType.X,
                                        op=mybir.AluOpType.min)

                # ---- maxes ----
                nc.vector.tensor_add(tmp[:], segMax[:],
                                     negMnot_sb[:, mslice])
                maxS = small.tile([128, P], f32, tag="maxS")
                nc.vector.tensor_reduce(
                    maxS[:], tmp.rearrange("p (q c) -> p q c", q=P),
                    axis=mybir.AxisListType.X, op=mybir.AluOpType.max)
                nc.vector.tensor_add(tmp[:], segMax[:],
                                     negMat_sb[:, mslice])
                maxD4 = small.tile([128, P], f32, tag="maxD4")
                nc.vector.tensor_reduce(
                    maxD4[:], tmp.rearrange("p (q c) -> p q c", q=P),
                    axis=mybir.AxisListType.X, op=mybir.AluOpType.max)

                # ap_sasc = max at own part of maxS
                nc.vector.tensor_add(t4[:], maxS[:], pmask_sb[:, 8:12])
                nc.vector.tensor_reduce(out_sb[:, c0 + 0:c0 + 1], t4[:],
                                        axis=mybir.AxisListType.X,
                                        op=mybir.AluOpType.max)
                # ap_sadc = max at own part of maxD4
                nc.vector.tensor_add(t4[:], maxD4[:], pmask_sb[:, 8:12])
                nc.vector.tensor_reduce(out_sb[:, c0 + 2:c0 + 3], t4[:],
                                        axis=mybir.AxisListType.X,
                                        op=mybir.AluOpType.max)
                # ap_dasc = max at other parts of maxS
                nc.vector.tensor_add(t4[:], maxS[:], pmask_sb[:, 12:16])
                nc.vector.tensor_reduce(out_sb[:, c0 + 4:c0 + 5], t4[:],
                                        axis=mybir.AxisListType.X,
                                        op=mybir.AluOpType.max)

            nc.gpsimd.dma_start(out_d[:], out_sb[:])

    if not nc.is_finalized():
        nc.finalize()
    return nc


def kernel(**inputs):
    import concourse.bass_utils as bass_utils

    x = np.ascontiguousarray(
        np.asarray(inputs["inputs"], dtype=np.float32).reshape(N, D))
    targets = np.asarray(inputs["targets"]).astype(np.int64)
    t = np.repeat(targets, P)               # class per row
    parts = np.tile(np.arange(P), B)        # part per row

    sq = np.sum(x.astype(np.float64) ** 2, axis=1).astype(np.float32)

    # ---- column permutation: part-major, class segments dup-padded to S ----
    counts = np.bincount(targets, minlength=NCLS)
    S = int(np.max(counts))
    if S % 2:
        S += 1
    NSEG = P * NCLS
    W = NSEG * S

    cls_rows = [np.nonzero(targets == c)[0] for c in range(NCLS)]
    colperm = np.empty(W, dtype=np.int64)
    pos = 0
    for q in range(P):
        for c in range(NCLS):
            rows = cls_rows[c]
            padded = np.concatenate(
                [rows, np.full(S - len(rows), rows[0], dtype=np.int64)])
            colperm[pos:pos + S] = padded * P + q
            pos += S

    rhsT = np.ascontiguousarray(x[colperm].T)              # [128, W]
    sqc = np.ascontiguousarray(sq[colperm][None, :])       # [1, W]

    # ---- per-core tensors ----
    # row i of core k, row-block rb, lane j  -> global row k*RPC + rb*128 + j
    onehot = np.zeros((N, NCLS), dtype=np.float32)
    onehot[np.arange(N), t] = 1.0

    in_maps = []
    seg_at = None  # class mask per (row, q, c) layout: (q major, c minor)
    for k in range(N_CORES):
        rows = np.arange(k * RPC, (k + 1) * RPC)
        lhsT = np.ascontiguousarray((-2.0 * x[rows]).T)    # [128, 1024]

        oh = onehot[rows]                                   # [1024, 32]
        # tile across q: [1024, NSEG] with layout (q, c)
        ohq = np.tile(oh, (1, P))                           # [1024, 128]
        bigM = np.ascontiguousarray(
            (ohq * BIG).reshape(NRB, 128, NSEG)
            .transpose(1, 0, 2).reshape(128, NRB * NSEG).astype(np.float32))
        negMat = np.ascontiguousarray(
            (ohq * -BIG).reshape(NRB, 128, NSEG)
            .transpose(1, 0, 2).reshape(128, NRB * NSEG).astype(np.float32))
        negMnot = np.ascontiguousarray(
            ((1.0 - ohq) * -BIG).reshape(NRB, 128, NSEG)
            .transpose(1, 0, 2).reshape(128, NRB * NSEG).astype(np.float32))

        in_maps.append({
            "lhsT": lhsT, "rhsT": rhsT, "sqc": sqc,
            "bigM": bigM, "negMnot": negMnot, "negMat": negMat,
        })

    # part masks, identical for every row-block/core: lane j has part j%4
    ph = np.zeros((128, P), dtype=np.float32)
    ph[np.arange(128), np.arange(128) % P] = 1.0
    pmask = np.concatenate([
        ph * BIG,            # +BIG at own part
        (1.0 - ph) * BIG,    # +BIG at other parts
        (1.0 - ph) * -BIG,   # -BIG at other parts (keep own)
        ph * -BIG,           # -BIG at own part   (keep others)
    ], axis=1).astype(np.float32)
    pmask = np.ascontiguousarray(pmask)
    ones = np.ones((1, 128), dtype=np.float32)
    for m in in_maps:
        m["pmask"] = pmask
        m["ones"] = ones

    nc = _build_bass(S, W)
    res = bass_utils.run_bass_kernel_spmd(nc, in_maps,
                                          list(range(N_CORES)))

    # ---- gather + host epilogue ----
    v = np.stack([np.asarray(res.results[k]["out"]) for k in range(N_CORES)])
    # v: [8, 128, NRB*5] ; row = k*RPC + rb*128 + lane
    v = v.reshape(N_CORES, 128, NRB, 5).transpose(0, 2, 1, 3).reshape(N, 5)

    d = np.sqrt(np.clip(sq[:, None].astype(np.float64) + v, EPS, None))
    ap_sasc, an_sasc, ap_sadc, an_sadc, ap_dasc = (
        d[:, 0], d[:, 1], d[:, 2], d[:, 3], d[:, 4])

    def hinge(an, ap, margin):
        return np.mean(np.maximum(0.0, margin - (an - ap)))

    loss = (hinge(an_sasc, ap_sasc, MARGIN_SASC)
            + hinge(an_sadc, ap_sadc, MARGIN_SADC)
            + hinge(an_sadc, ap_dasc, MARGIN_DASC))
    return np.float32(loss)


# revision 12
# speedup vs baseline: 1.7416x; 1.7416x over previous
import numpy as np

B, P, D = 2048, 4, 128
N = B * P
NCLS = 32
N_CORES = 8
RPC = N // N_CORES          # 1024 rows per core
NRB = RPC // 128            # 8 row-blocks of 128
BIG = 4096.0                # fp16-safe mask offset (|val| < 1024)
EPS = 1e-12
MARGIN_SASC = 0.3
MARGIN_SADC = 0.2
MARGIN_DASC = 0.4
SQ_CENTER = 128.0


def _build_bass(S, W):
    import concourse.bass as bass
    from concourse import bacc, tile
    from concourse import mybir

    nc = bacc.Bacc("TRN2", target_bir_lowering=False, debug=False,
                   num_devices=N_CORES)

    f32 = mybir.dt.float32
    f16 = mybir.dt.float16

    lhsT_d = nc.dram_tensor("lhsT", [D, RPC], f16, kind="ExternalInput").ap()
    rhsT_d = nc.dram_tensor("rhsT", [D, W], f16, kind="ExternalInput").ap()
    sqc_d = nc.dram_tensor("sqc", [1, W], f16, kind="ExternalInput").ap()
    bigM_d = nc.dram_tensor("bigM", [128, NRB * NCLS * P], f16,
                            kind="ExternalInput").ap()
    negMnot_d = nc.dram_tensor("negMnot", [128, NRB * NCLS * P], f16,
                               kind="ExternalInput").ap()
    negMat_d = nc.dram_tensor("negMat", [128, NRB * NCLS * P], f16,
                              kind="ExternalInput").ap()
    pmask_d = nc.dram_tensor("pmask", [128, 16], f16,
                             kind="ExternalInput").ap()
    ones_d = nc.dram_tensor("ones", [1, 128], f16,
                            kind="ExternalInput").ap()
    out_d = nc.dram_tensor("out", [128, NRB * 5], f16,
                           kind="ExternalOutput").ap()

    NSEG = P * NCLS          # 128 column segments of size S
    kmax = 512 // S          # segments per PSUM bank (fp32 PSUM = 512/part)
    n_tiles = (NSEG + kmax - 1) // kmax

    from contextlib import ExitStack
    with tile.TileContext(nc) as tc:
        with ExitStack() as ctx:
            const = ctx.enter_context(tc.tile_pool(name="const", bufs=1))
            valp = ctx.enter_context(tc.tile_pool(name="val", bufs=2))
            psum = ctx.enter_context(
                tc.tile_pool(name="psum", bufs=8, space="PSUM"))
            small = ctx.enter_context(tc.tile_pool(name="small", bufs=2))

            lhsT_sb = const.tile([D, RPC], f16, tag="lhsT")
            rhsT_sb = const.tile([D, W], f16, tag="rhsT")
            sqc_sb = const.tile([1, W], f16, tag="sqc")
            bigM_sb = const.tile([128, NRB * NSEG], f16, tag="bigM")
            negMnot_sb = const.tile([128, NRB * NSEG], f16, tag="negMnot")
            negMat_sb = const.tile([128, NRB * NSEG], f16, tag="negMat")
            pmask_sb = const.tile([128, 16], f16, tag="pmask")
            ones_sb = const.tile([1, 128], f16, tag="ones")
            out_sb = const.tile([128, NRB * 5], f16, tag="out")

            nc.gpsimd.dma_start(lhsT_sb[:], lhsT_d[:])
            nc.gpsimd.dma_start(rhsT_sb[:], rhsT_d[:])
            nc.gpsimd.dma_start(sqc_sb[:], sqc_d[:])
            nc.gpsimd.dma_start(bigM_sb[:], bigM_d[:])
            nc.gpsimd.dma_start(negMnot_sb[:], negMnot_d[:])
            nc.gpsimd.dma_start(negMat_sb[:], negMat_d[:])
            nc.gpsimd.dma_start(pmask_sb[:], pmask_d[:])
            nc.gpsimd.dma_start(ones_sb[:], ones_d[:])

            for rb in range(NRB):
                val = valp.tile([128, NSEG * S], f16, tag="val")
                val3 = val.rearrange("p (g s) -> p g s", g=NSEG)
                lhs_rb = lhsT_sb[:, rb * 128:(rb + 1) * 128]
                for t in range(n_tiles):
                    seg0 = t * kmax
                    k = min(kmax, NSEG - seg0)
                    nf = k * S
                    ps = psum.tile([128, kmax * S], f32, tag="ps",
                                   space="PSUM")
                    nc.tensor.matmul(ps[:, :nf], lhs_rb,
                                     rhsT_sb[:, seg0 * S:seg0 * S + nf],
                                     start=True, stop=False)
                    nc.tensor.matmul(ps[:, :nf], ones_sb,
                                     sqc_sb[:, seg0 * S:seg0 * S + nf],
                                     start=False, stop=True)
                    dst = val[:, seg0 * S:seg0 * S + nf]
                    if t % 4 == 3:
                        nc.vector.tensor_copy(dst, ps[:, :nf])
                    else:
                        nc.scalar.copy(dst, ps[:, :nf])

                # segment reductions: [128, NSEG, S] -> [128, NSEG]
                segMin = small.tile([128, NSEG], f16, tag="segMin")
                segMax = small.tile([128, NSEG], f16, tag="segMax")
                nc.vector.tensor_reduce(segMin[:], val3,
                                        axis=mybir.AxisListType.X,
                                        op=mybir.AluOpType.min)
                nc.vector.tensor_reduce(segMax[:], val3,
                                        axis=mybir.AxisListType.X,
                                        op=mybir.AluOpType.max)

                mslice = slice(rb * NSEG, (rb + 1) * NSEG)
                sm3 = segMin.rearrange("p (q c) -> p q c", q=P)

                # ---- mins ----
                tmp = small.tile([128, NSEG], f16, tag="tmp")
                nc.vector.tensor_add(tmp[:], segMin[:], bigM_sb[:, mslice])
                minD = small.tile([128, P], f16, tag="minD")
                nc.vector.tensor_reduce(
                    minD[:], tmp.rearrange("p (q c) -> p q c", q=P),
                    axis=mybir.AxisListType.X, op=mybir.AluOpType.min)
                minAll = small.tile([128, P], f16, tag="minAll")
                nc.vector.tensor_reduce(minAll[:], sm3,
                                        axis=mybir.AxisListType.X,
                                        op=mybir.AluOpType.min)

                t4 = small.tile([128, P], f16, tag="t4")
                c0 = rb * 5
                # an_sadc = min over q!=p of minD  (add +BIG at own part)
                nc.vector.tensor_add(t4[:], minD[:], pmask_sb[:, 0:4])
                nc.vector.tensor_reduce(out_sb[:, c0 + 3:c0 + 4], t4[:],
                                        axis=mybir.AxisListType.X,
                                        op=mybir.AluOpType.min)
                # an_sasc = min( minD at own part , minAll at other parts )
                t4b = small.tile([128, P], f16, tag="t4b")
                nc.vector.tensor_add(t4[:], minD[:], pmask_sb[:, 4:8])
                nc.vector.tensor_add(t4b[:], minAll[:], pmask_sb[:, 0:4])
                nc.vector.tensor_tensor(t4[:], t4[:], t4b[:],
                                        op=mybir.AluOpType.min)
                nc.vector.tensor_reduce(out_sb[:, c0 + 1:c0 + 2], t4[:],
                                        axis=mybir.AxisListType.X,
                                        op=mybir.AluOpType.min)

                # ---- maxes ----
                nc.vector.tensor_add(tmp[:], segMax[:],
                                     negMnot_sb[:, mslice])
                maxS = small.tile([128, P], f16, tag="maxS")
                nc.vector.tensor_reduce(
                    maxS[:], tmp.rearrange("p (q c) -> p q c", q=P),
                    axis=mybir.AxisListType.X, op=mybir.AluOpType.max)
                nc.vector.tensor_add(tmp[:], segMax[:],
                                     negMat_sb[:, mslice])
                maxD4 = small.tile([128, P], f16, tag="maxD4")
                nc.vector.tensor_reduce(
                    maxD4[:], tmp.rearrange("p (q c) -> p q c", q=P),
                    axis=mybir.AxisListType.X, op=mybir.AluOpType.max)

                # ap_sasc = max at own part of maxS
                nc.vector.tensor_add(t4[:], maxS[:], pmask_sb[:, 8:12])
                nc.vector.tensor_reduce(out_sb[:, c0 + 0:c0 + 1], t4[:],
                                        axis=mybir.AxisListType.X,
                                        op=mybir.AluOpType.max)
                # ap_sadc = max at own part of maxD4
                nc.vector.tensor_add(t4[:], maxD4[:], pmask_sb[:, 8:12])
                nc.vector.tensor_reduce(out_sb[:, c0 + 2:c0 + 3], t4[:],
                                        axis=mybir.AxisListType.X,
                                        op=mybir.AluOpType.max)
                # ap_dasc = max at other parts of maxS
                nc.vector.tensor_add(t4[:], maxS[:], pmask_sb[:, 12:16])
                nc.vector.tensor_reduce(out_sb[:, c0 + 4:c0 + 5], t4[:],
                                        axis=mybir.AxisListType.X,
                                        op=mybir.AluOpType.max)

            nc.gpsimd.dma_start(out_d[:], out_sb[:])

    if not nc.is_finalized():
        nc.finalize()
    return nc


def kernel(**inputs):
    import concourse.bass_utils as bass_utils

    x = np.ascontiguousarray(
        np.asarray(inputs["inputs"], dtype=np.float32).reshape(N, D))
    targets = np.asarray(inputs["targets"]).astype(np.int64)
    t = np.repeat(targets, P)               # class per row

    sq = np.sum(x.astype(np.float64) ** 2, axis=1)

    # ---- column permutation: part-major, class segments dup-padded to S ----
    counts = np.bincount(targets, minlength=NCLS)
    S = int(np.max(counts))
    if S % 2:
        S += 1
    NSEG = P * NCLS
    W = NSEG * S

    cls_rows = [np.nonzero(targets == c)[0] for c in range(NCLS)]
    colperm = np.empty(W, dtype=np.int64)
    pos = 0
    for q in range(P):
        for c in range(NCLS):
            rows = cls_rows[c]
            padded = np.concatenate(
                [rows, np.full(S - len(rows), rows[0], dtype=np.int64)])
            colperm[pos:pos + S] = padded * P + q
            pos += S

    x16 = x.astype(np.float16)
    rhsT = np.ascontiguousarray(x16[colperm].T)             # [128, W] f16
    sqc = np.ascontiguousarray(
        (sq - SQ_CENTER).astype(np.float16)[colperm][None, :])

    # ---- per-core tensors ----
    onehot = np.zeros((N, NCLS), dtype=np.float16)
    onehot[np.arange(N), t] = 1.0

    in_maps = []
    for k in range(N_CORES):
        rows = np.arange(k * RPC, (k + 1) * RPC)
        lhsT = np.ascontiguousarray((-2.0 * x16[rows]).T)   # [128, 1024] f16

        oh = onehot[rows]                                    # [1024, 32]
        ohq = np.tile(oh, (1, P))                            # [1024, 128]

        def pack(m):
            return np.ascontiguousarray(
                m.reshape(NRB, 128, NSEG).transpose(1, 0, 2)
                .reshape(128, NRB * NSEG).astype(np.float16))

        in_maps.append({
            "lhsT": lhsT, "rhsT": rhsT, "sqc": sqc,
            "bigM": pack(ohq * np.float16(BIG)),
            "negMat": pack(ohq * np.float16(-BIG)),
            "negMnot": pack((1.0 - ohq) * np.float16(BIG) * np.float16(-1)),
        })

    # part masks, identical for every row-block/core: lane j has part j%4
    ph = np.zeros((128, P), dtype=np.float16)
    ph[np.arange(128), np.arange(128) % P] = 1.0
    pmask = np.ascontiguousarray(np.concatenate([
        ph * np.float16(BIG),            # +BIG at own part
        (1.0 - ph) * np.float16(BIG),    # +BIG at other parts
        (1.0 - ph) * np.float16(-BIG),   # -BIG at other parts (keep own)
        ph * np.float16(-BIG),           # -BIG at own part   (keep others)
    ], axis=1).astype(np.float16))
    ones = np.ones((1, 128), dtype=np.float16)
    for m in in_maps:
        m["pmask"] = pmask
        m["ones"] = ones

    nc = _build_bass(S, W)
    res = bass_utils.run_bass_kernel_spmd(nc, in_maps,
                                          list(range(N_CORES)))

    # ---- gather + host epilogue ----
    v = np.stack([np.asarray(res.results[k]["out"]) for k in range(N_CORES)])
    # v: [8, 128, NRB*5] ; row = k*RPC + rb*128 + lane
    v = v.astype(np.float64).reshape(N_CORES, 128, NRB, 5)
    v = v.transpose(0, 2, 1, 3).reshape(N, 5)

    d = np.sqrt(np.clip(sq[:, None] + SQ_CENTER + v, EPS, None))
    ap_sasc, an_sasc, ap_sadc, an_sadc, ap_dasc = (
        d[:, 0], d[:, 1], d[:, 2], d[:, 3], d[:, 4])

    def hinge(an, ap, margin):
        return np.mean(np.maximum(0.0, margin - (an - ap)))

    loss = (hinge(an_sasc, ap_sasc, MARGIN_SASC)
            + hinge(an_sadc, ap_sadc, MARGIN_SADC)
            + hinge(an_sadc, ap_dasc, MARGIN_DASC))
    return np.float32(loss)
